# revision 10
# baseline (speedup 1.0000x reference)
"""CTC loss kernel for Trainium2 (8 NeuronCores, data-parallel over batch).

Math: with raw logits G[b,t,s] = pred[b,t,ext[b,s]] (ext = blank-interleaved
targets) the CTC forward recursion commutes with the per-frame log-softmax
normalizer: running the recursion on raw logits and subtracting
sum_t logsumexp_c(pred[b,t,:]) at the end gives the same loss. So the chip
computes (1) sum_c exp(pred) per (b,t) via streaming ACT exp+accumulate
(the memory-bound bulk) and (2) a probability-space forward recursion on the
VectorEngine with periodic renormalization; the recorded renorm multipliers
are compensated exactly on the host, which finishes the tiny scalar math in
float64.
"""

import sys

sys.path.insert(0, "/opt/trn_rl_repo")

import numpy as np

import concourse.bacc as bacc
import concourse.tile as tile
from concourse import mybir
from concourse.bass_utils import run_bass_kernel_spmd

B, T, C, L = 128, 160, 6625, 25
S = 2 * L + 1  # 51 CTC states
N_CORES = 8
BS = B // N_CORES  # 16 samples per core
TBLK = 8  # t-values per 128-row streaming block (8*16 = 128 rows)
NBLK = T // TBLK  # 20
CHUNKS = [(0, 1657), (1657, 3313), (3313, 4969), (4969, 6625)]
NCH = len(CHUNKS)
CHMAX = max(c1 - c0 for c0, c1 in CHUNKS)
NEG = -1.0e4  # exp() underflows to exactly 0.0f
RENORM_EVERY = 8
REN_STEPS = [t for t in range(1, T) if t % RENORM_EVERY == 0]
NREN = len(REN_STEPS)  # 19

f32 = mybir.dt.float32
f16 = mybir.dt.float16
Exp = mybir.ActivationFunctionType.Exp

_CACHE = {}


def _build_program():
    if "nc" in _CACHE:
        return _CACHE["nc"]
    nc = bacc.Bacc("TRN2", target_bir_lowering=False, debug=False,
                   num_devices=N_CORES)
    pred_d = nc.dram_tensor("pred", [BS, T, C], f32, kind="ExternalInput").ap()
    g_d = nc.dram_tensor("g", [BS, T * S], f32, kind="ExternalInput").ap()
    skip_d = nc.dram_tensor("skip", [BS, S], f32, kind="ExternalInput").ap()
    acc_d = nc.dram_tensor("acc", [NBLK, 128, NCH], f32,
                           kind="ExternalOutput").ap()
    afin_d = nc.dram_tensor("afin", [BS, S], f32, kind="ExternalOutput").ap()
    rnorm_d = nc.dram_tensor("rnorm", [BS, NREN], f32,
                             kind="ExternalOutput").ap()

    with tile.TileContext(nc) as tc:
        with (
            tc.tile_pool(name="persist", bufs=1) as pp,
            tc.tile_pool(name="steps", bufs=2) as stepp,
            tc.tile_pool(name="stream", bufs=3) as spool,
            tc.tile_pool(name="accp", bufs=3) as apool,
        ):
            # ---- recursion inputs (small), emitted first so ACT/DVE start early
            gt = pp.tile([BS, T * S], f32, tag="gt")
            pt = pp.tile([BS, T * S], f32, tag="pt")
            skipt = pp.tile([BS, S], f32, tag="skipt")
            half = (T // 2) * S
            nc.sync.dma_start(out=gt[:, :half], in_=g_d[:, :half])
            nc.sync.dma_start(out=gt[:, half:], in_=g_d[:, half:])
            nc.sync.dma_start(out=skipt[:], in_=skip_d[:])
            nc.scalar.activation(pt[:, :half], gt[:, :half], Exp)
            nc.scalar.activation(pt[:, half:], gt[:, half:], Exp)

            # ---- probability-space forward recursion, DVE only.
            # A tiles have 2 guard columns (always 0) so the s-1 / s-2 shifted
            # reads come from plain offset APs.
            Aa = pp.tile([BS, S + 2], f32, tag="Aa")
            Ab = pp.tile([BS, S + 2], f32, tag="Ab")
            Mt = pp.tile([BS, NREN], f32, tag="Mt")
            nc.vector.memset(Aa[:], 0.0)
            nc.vector.memset(Ab[:, 0:2], 0.0)
            # alpha0: states 0,1 get p[t=0, s=0,1], rest 0
            nc.vector.tensor_copy(out=Aa[:, 2:4], in_=pt[:, 0:2])

            cur, nxt = Aa, Ab
            k = 0
            for t in range(1, T):
                base = t * S
                u = stepp.tile([BS, S], f32, tag="u")
                v = stepp.tile([BS, S], f32, tag="v")
                # u = A[s] + A[s-1]
                nc.vector.tensor_add(out=u[:], in0=cur[:, 2:S + 2],
                                     in1=cur[:, 1:S + 1])
                # v = A[s-2] * skip_ok[s]
                nc.vector.tensor_mul(out=v[:], in0=cur[:, 0:S], in1=skipt[:])
                nc.vector.tensor_add(out=u[:], in0=u[:], in1=v[:])
                # A_new[s] = p_t[s] * (sum of paths)
                nc.vector.tensor_mul(out=nxt[:, 2:S + 2], in0=u[:],
                                     in1=pt[:, base:base + S])
                if t % RENORM_EVERY == 0:
                    mx = stepp.tile([BS, 1], f32, tag="mx")
                    nc.vector.reduce_max(mx[:], nxt[:, 2:S + 2],
                                         axis=mybir.AxisListType.X)
                    # record the actual multiplier used; host compensates with
                    # -log(r), so reciprocal accuracy does not matter.
                    nc.vector.reciprocal(out=Mt[:, k:k + 1], in_=mx[:])
                    nc.vector.tensor_scalar_mul(out=nxt[:, 2:S + 2],
                                                in0=nxt[:, 2:S + 2],
                                                scalar1=Mt[:, k:k + 1])
                    k += 1
                cur, nxt = nxt, cur
            assert k == NREN
            nc.sync.dma_start(out=afin_d[:], in_=cur[:, 2:S + 2])
            nc.sync.dma_start(out=rnorm_d[:], in_=Mt[:])

            # ---- streaming sum(exp(pred)) over C, 128 (b,t) rows per block.
            # The paired NeuronCore shares the 16 SBUF AXI ports (435 GB/s per
            # SEngine), capping plain fp32 streaming at ~217 GB/s/core. The
            # SWDGE inline fp32->fp16 cast halves the SBUF-write traffic, so
            # the HBM read side (~358 GB/s/core share) binds instead. exp is
            # computed in-place (elementwise); accumulation stays fp32.
            for j in range(NBLK):
                acc_t = apool.tile([128, NCH], f32, tag="acc")
                ct = spool.tile([128, C], f16, tag="chunk")
                src = pred_d[:, j * TBLK:(j + 1) * TBLK, :]
                nc.gpsimd.dma_start(out=ct[:], in_=src)
                for ci, (c0, c1) in enumerate(CHUNKS):
                    nc.scalar.activation(ct[:, c0:c1], ct[:, c0:c1], Exp,
                                         accum_out=acc_t[:, ci:ci + 1])
                nc.sync.dma_start(out=acc_d[j], in_=acc_t[:])

    nc.compile()
    _CACHE["nc"] = nc
    return nc


def prepare_in_maps(pred, targets, lens):
    """Host prep: extended labels, gathered logits G, skip mask; shard by core."""
    ext = np.zeros((B, S), dtype=np.int64)
    ext[:, 1::2] = targets
    G = pred[np.arange(B)[:, None, None], np.arange(T)[None, :, None],
             ext[:, None, :]]  # [B, T, S]
    valid = np.arange(S)[None, :] < (2 * lens + 1)[:, None]  # [B, S]
    G = np.where(valid[:, None, :], G, NEG).astype(np.float32)
    skip = np.pad((ext[:, 2:] != ext[:, :-2]) & (ext[:, 2:] != 0),
                  ((0, 0), (2, 0))).astype(np.float32)
    in_maps = []
    for c in range(N_CORES):
        sl = slice(c * BS, (c + 1) * BS)
        in_maps.append({
            "pred": np.ascontiguousarray(pred[sl]),
            "g": np.ascontiguousarray(G[sl].reshape(BS, T * S)),
            "skip": np.ascontiguousarray(skip[sl]),
        })
    return in_maps


def finish_host(results, lens):
    """Combine per-core outputs into the scalar mean loss (float64)."""
    loss_b = np.zeros(B, dtype=np.float64)
    with np.errstate(divide="ignore", invalid="ignore"):
        for c in range(N_CORES):
            r = results[c]
            acc = r["acc"].astype(np.float64)  # [NBLK, 128, NCH]
            lse = np.log(acc.sum(-1))  # [NBLK, 128]; row p = b*8 + t_off
            s_lse = lse.reshape(NBLK, BS, TBLK).sum((0, 2))  # [BS]
            afin = r["afin"].astype(np.float64)  # [BS, S]
            rn = r["rnorm"].astype(np.float64)  # [BS, NREN]
            log_carry = np.log(rn).sum(1)  # [BS]
            for b in range(BS):
                gb = c * BS + b
                sE = 2 * int(lens[gb])
                le = np.logaddexp(np.log(afin[b, sE]), np.log(afin[b, sE - 1]))
                loss_b[gb] = s_lse[b] + log_carry[b] - le
    loss_b = np.where(loss_b >= 1e29, 0.0, loss_b)
    loss_b = np.where(np.isfinite(loss_b), loss_b, 0.0)
    loss = np.mean(loss_b / np.maximum(lens.astype(np.float64), 1.0))
    return np.float32(loss)


def kernel(pred, targets, targets_lengths):
    pred = np.asarray(pred, dtype=np.float32)
    targets = np.asarray(targets).astype(np.int64)
    lens = np.asarray(targets_lengths).astype(np.int64)

    nc = _build_program()
    in_maps = prepare_in_maps(pred, targets, lens)
    res = run_bass_kernel_spmd(nc, in_maps, core_ids=list(range(N_CORES)))
    return finish_host(res.results, lens)


# revision 13
# speedup vs baseline: 1.1608x; 1.1608x over previous
"""CTC loss kernel for Trainium2 (8 NeuronCores, data-parallel over batch).

Math: with raw logits G[b,t,s] = pred[b,t,ext[b,s]] (ext = blank-interleaved
targets) the CTC forward recursion commutes with the per-frame log-softmax
normalizer: running the recursion on raw logits and subtracting
sum_t logsumexp_c(pred[b,t,:]) at the end gives the same loss. So the chip
computes (1) sum_c exp(pred) per (b,t) via streaming ACT exp+accumulate
(the memory-bound bulk) and (2) a probability-space forward recursion on the
VectorEngine with periodic renormalization; the recorded renorm multipliers
are compensated exactly on the host, which finishes the tiny scalar math in
float64.
"""

import sys

sys.path.insert(0, "/opt/trn_rl_repo")

import numpy as np

import concourse.bacc as bacc
import concourse.tile as tile
from concourse import mybir
from concourse.bass_utils import run_bass_kernel_spmd

B, T, C, L = 128, 160, 6625, 25
S = 2 * L + 1  # 51 CTC states
N_CORES = 8
BS = B // N_CORES  # 16 samples per core
TBLK = 8  # t-values per 128-row streaming block (8*16 = 128 rows)
NBLK = T // TBLK  # 20
CHUNKS = [(0, 1657), (1657, 3313), (3313, 4969), (4969, 6625)]
NCH = len(CHUNKS)
CHMAX = max(c1 - c0 for c0, c1 in CHUNKS)
NEG = -1.0e4  # exp() underflows to exactly 0.0f
RENORM_EVERY = 8
REN_STEPS = [t for t in range(1, T) if t % RENORM_EVERY == 0]
NREN = len(REN_STEPS)  # 19

f32 = mybir.dt.float32
f16 = mybir.dt.float16
Exp = mybir.ActivationFunctionType.Exp

_CACHE = {}


def _build_program():
    if "nc" in _CACHE:
        return _CACHE["nc"]
    nc = bacc.Bacc("TRN2", target_bir_lowering=False, debug=False,
                   num_devices=N_CORES)
    pred_d = nc.dram_tensor("pred", [BS, T, C], f32, kind="ExternalInput").ap()
    g_d = nc.dram_tensor("g", [BS, T * S], f32, kind="ExternalInput").ap()
    skip_d = nc.dram_tensor("skip", [BS, S], f32, kind="ExternalInput").ap()
    acc_d = nc.dram_tensor("acc", [NBLK, 128, NCH], f32,
                           kind="ExternalOutput").ap()
    afin_d = nc.dram_tensor("afin", [BS, S], f32, kind="ExternalOutput").ap()
    rnorm_d = nc.dram_tensor("rnorm", [BS, NREN], f32,
                             kind="ExternalOutput").ap()

    with tile.TileContext(nc) as tc:
        with (
            tc.tile_pool(name="persist", bufs=1) as pp,
            tc.tile_pool(name="steps", bufs=2) as stepp,
            tc.tile_pool(name="stream", bufs=4) as spool,
            tc.tile_pool(name="accp", bufs=3) as apool,
        ):
            # ---- recursion inputs (small), emitted first so ACT/DVE start early
            gt = pp.tile([BS, T * S], f32, tag="gt")
            pt = pp.tile([BS, T * S], f32, tag="pt")
            skipt = pp.tile([BS, S], f32, tag="skipt")
            half = (T // 2) * S
            nc.sync.dma_start(out=gt[:, :half], in_=g_d[:, :half])
            nc.sync.dma_start(out=gt[:, half:], in_=g_d[:, half:])
            nc.sync.dma_start(out=skipt[:], in_=skip_d[:])
            nc.scalar.activation(pt[:, :half], gt[:, :half], Exp)
            nc.scalar.activation(pt[:, half:], gt[:, half:], Exp)

            # ---- probability-space forward recursion, DVE only.
            # A tiles have 2 guard columns (always 0) so the s-1 / s-2 shifted
            # reads come from plain offset APs.
            Aa = pp.tile([BS, S + 2], f32, tag="Aa")
            Ab = pp.tile([BS, S + 2], f32, tag="Ab")
            Mt = pp.tile([BS, NREN], f32, tag="Mt")
            nc.vector.memset(Aa[:], 0.0)
            nc.vector.memset(Ab[:, 0:2], 0.0)
            # alpha0: states 0,1 get p[t=0, s=0,1], rest 0
            nc.vector.tensor_copy(out=Aa[:, 2:4], in_=pt[:, 0:2])

            cur, nxt = Aa, Ab
            k = 0
            for t in range(1, T):
                base = t * S
                u = stepp.tile([BS, S], f32, tag="u")
                v = stepp.tile([BS, S], f32, tag="v")
                # u = A[s] + A[s-1]
                nc.vector.tensor_add(out=u[:], in0=cur[:, 2:S + 2],
                                     in1=cur[:, 1:S + 1])
                # v = A[s-2] * skip_ok[s]
                nc.vector.tensor_mul(out=v[:], in0=cur[:, 0:S], in1=skipt[:])
                nc.vector.tensor_add(out=u[:], in0=u[:], in1=v[:])
                # A_new[s] = p_t[s] * (sum of paths)
                nc.vector.tensor_mul(out=nxt[:, 2:S + 2], in0=u[:],
                                     in1=pt[:, base:base + S])
                if t % RENORM_EVERY == 0:
                    mx = stepp.tile([BS, 1], f32, tag="mx")
                    nc.vector.reduce_max(mx[:], nxt[:, 2:S + 2],
                                         axis=mybir.AxisListType.X)
                    # record the actual multiplier used; host compensates with
                    # -log(r), so reciprocal accuracy does not matter.
                    nc.vector.reciprocal(out=Mt[:, k:k + 1], in_=mx[:])
                    nc.vector.tensor_scalar_mul(out=nxt[:, 2:S + 2],
                                                in0=nxt[:, 2:S + 2],
                                                scalar1=Mt[:, k:k + 1])
                    k += 1
                cur, nxt = nxt, cur
            assert k == NREN
            nc.sync.dma_start(out=afin_d[:], in_=cur[:, 2:S + 2])
            nc.sync.dma_start(out=rnorm_d[:], in_=Mt[:])

            # ---- streaming sum(exp(pred)) over C, 128 (b,t) rows per block.
            # The paired NeuronCore shares the 16 SBUF AXI ports (435 GB/s per
            # SEngine), capping plain fp32 streaming at ~217 GB/s/core. The
            # SWDGE inline fp32->fp16 cast halves the SBUF-write traffic, so
            # the HBM read side (~358 GB/s/core share) binds instead. exp is
            # computed in-place (elementwise); accumulation stays fp32.
            for j in range(NBLK):
                acc_t = apool.tile([128, NCH], f32, tag="acc")
                src = pred_d[:, j * TBLK:(j + 1) * TBLK, :]
                if j in (0, NBLK - 1):
                    # chunked load into separate tiles for the first/last
                    # block: lets ACT start before the full block lands
                    # (first) and shortens the exposed ACT tail after the
                    # final transfer (last). Deps are tile-granular.
                    for ci, (c0, c1) in enumerate(CHUNKS):
                        cp = spool.tile([128, CHMAX], f16, tag="chunkpart")
                        w = c1 - c0
                        nc.gpsimd.dma_start(out=cp[:, :w],
                                            in_=src[:, :, c0:c1])
                        nc.scalar.activation(cp[:, :w], cp[:, :w], Exp,
                                             accum_out=acc_t[:, ci:ci + 1])
                else:
                    ct = spool.tile([128, C], f16, tag="chunk")
                    nc.gpsimd.dma_start(out=ct[:], in_=src)
                    for ci, (c0, c1) in enumerate(CHUNKS):
                        nc.scalar.activation(ct[:, c0:c1], ct[:, c0:c1], Exp,
                                             accum_out=acc_t[:, ci:ci + 1])
                nc.sync.dma_start(out=acc_d[j], in_=acc_t[:])

    nc.compile()
    _CACHE["nc"] = nc
    return nc


def prepare_in_maps(pred, targets, lens):
    """Host prep: extended labels, gathered logits G, skip mask; shard by core."""
    ext = np.zeros((B, S), dtype=np.int64)
    ext[:, 1::2] = targets
    G = pred[np.arange(B)[:, None, None], np.arange(T)[None, :, None],
             ext[:, None, :]]  # [B, T, S]
    valid = np.arange(S)[None, :] < (2 * lens + 1)[:, None]  # [B, S]
    G = np.where(valid[:, None, :], G, NEG).astype(np.float32)
    skip = np.pad((ext[:, 2:] != ext[:, :-2]) & (ext[:, 2:] != 0),
                  ((0, 0), (2, 0))).astype(np.float32)
    in_maps = []
    for c in range(N_CORES):
        sl = slice(c * BS, (c + 1) * BS)
        in_maps.append({
            "pred": np.ascontiguousarray(pred[sl]),
            "g": np.ascontiguousarray(G[sl].reshape(BS, T * S)),
            "skip": np.ascontiguousarray(skip[sl]),
        })
    return in_maps


def finish_host(results, lens):
    """Combine per-core outputs into the scalar mean loss (float64)."""
    loss_b = np.zeros(B, dtype=np.float64)
    with np.errstate(divide="ignore", invalid="ignore"):
        for c in range(N_CORES):
            r = results[c]
            acc = r["acc"].astype(np.float64)  # [NBLK, 128, NCH]
            lse = np.log(acc.sum(-1))  # [NBLK, 128]; row p = b*8 + t_off
            s_lse = lse.reshape(NBLK, BS, TBLK).sum((0, 2))  # [BS]
            afin = r["afin"].astype(np.float64)  # [BS, S]
            rn = r["rnorm"].astype(np.float64)  # [BS, NREN]
            log_carry = np.log(rn).sum(1)  # [BS]
            for b in range(BS):
                gb = c * BS + b
                sE = 2 * int(lens[gb])
                le = np.logaddexp(np.log(afin[b, sE]), np.log(afin[b, sE - 1]))
                loss_b[gb] = s_lse[b] + log_carry[b] - le
    loss_b = np.where(loss_b >= 1e29, 0.0, loss_b)
    loss_b = np.where(np.isfinite(loss_b), loss_b, 0.0)
    loss = np.mean(loss_b / np.maximum(lens.astype(np.float64), 1.0))
    return np.float32(loss)


def kernel(pred, targets, targets_lengths):
    pred = np.asarray(pred, dtype=np.float32)
    targets = np.asarray(targets).astype(np.int64)
    lens = np.asarray(targets_lengths).astype(np.int64)

    nc = _build_program()
    in_maps = prepare_in_maps(pred, targets, lens)
    res = run_bass_kernel_spmd(nc, in_maps, core_ids=list(range(N_CORES)))
    return finish_host(res.results, lens)
